# revision 5
# baseline (speedup 1.0000x reference)
"""Trainium2 Bass kernel for nn_DualWeightAttention (B=2, S=2048, H=2048, 16 heads).

Sharding: tensor-parallel over heads — 2 heads per core on 8 cores.
Each core computes q/k/v projections for its 2 heads, attention for those
heads (both batches), and a partial output projection against its 256-row
slice of Wo.T. The 8 partial [4096, 2048] fp16 outputs are summed on the host.

On-chip layouts (per core), fp16 operands (same PE rate as bf16, 8x finer
mantissa; every tensor here is O(1)-scaled so range is ample):
  qT, kT  [128(d), head, B*S]  fp16  (head dim on partitions)
  v       [128(s), tile, 256]  fp16  (seq on partitions)
  scoresT [128(k), q]          psum f32 = (kT k-tile).T @ qT chunk
  attn_u  [128(k), kt, 512]    fp16  = exp(scoresT) * exp(maskT)   (host
                                       precomputes exp(mask); ScalarE's exp
                                       evacuates PSUM; the mask factor is an
                                       fp16 SBUF*SBUF multiply split across
                                       DVE and GpSimd so neither paces)
  asum8   [128(k), 8, 512]     fp16  = attn_u[:, 0:8] + attn_u[:, 8:16] (DVE)
  uT      [128(d), head, S]    fp16  = ((attn_u @ v)^T) * 1/denom
  out     [128(s), 2048]       fp16  = uT.T @ WoT-slice (2-head accumulate)

Softmax is unnormalized: the denominator is a DVE pairwise add (halves the
PE ones-matmul stream) followed by 8 accumulating ones-vector matmuls,
1/denom via a DVE approx reciprocal broadcast across partitions on GpSimd,
applied while evacuating the PV accumulator.

Phase-2 emission is software-pipelined at k-tile granularity: period i
interleaves QK(i+1) matmuls with PV(i) matmuls, the denominator matmuls of
unit i (kt 2..5), and the out-projection tiles of the previously finished
q-chunk pair (kt 6..13, half a pair per unit) — the PE never sits behind
the reciprocal/normalize chain and stays ~95% busy.

DMA: all DRAM tensors are host-pre-tiled so every descriptor is a 2-8 KiB
contiguous run (hsT [sc,qf,p,ko,t], mask [b,qq,half,p,kt,q], out
[st,p,j]); output is stored fp16 (error ~5e-4, far inside the 2e-2 gate).
"""

import numpy as np

import concourse.mybir as mybir
import concourse.tile as tile
from concourse import bacc
from concourse.bass_utils import run_bass_kernel_spmd

P = 128
B = 2
S = 2048
H = 2048
NH = 16
HD = 128
NCORES = 8
HPC = NH // NCORES  # heads per core
DC = HPC * HD       # d-columns per core
QC = 512            # q-chunk (matmul moving free dim)
HT = H // P         # contraction tiles for projections
NQT = 4             # hsT quarters per s-chunk
KOQ = HT // NQT
SCALE = 1.0 / float(np.sqrt(HD))

F32 = mybir.dt.float32
DT16 = mybir.dt.float16

EXP = mybir.ActivationFunctionType.Exp
ADD = mybir.AluOpType.add
MULT = mybir.AluOpType.mult


def build_attention_nc(s=S):
    bs = B * s
    kt_n = s // P   # k tiles per batch
    nq = s // QC    # q chunks per batch
    st_n = s // P   # s tiles per batch (out projection)
    vt_n = bs // P  # v tiles (both batches)
    nsc = bs // QC  # s-chunks (projection)

    nc = bacc.Bacc("TRN2", target_bir_lowering=False, debug=False, num_devices=NCORES)
    # host-pre-tiled DRAM layouts: every per-partition run is contiguous
    hsT = nc.dram_tensor("hsT", [nsc, NQT, P, KOQ, QC], DT16, kind="ExternalInput")
    maskT = nc.dram_tensor("maskT", [B, nq, 2, P, kt_n // 2, QC], DT16,
                           kind="ExternalInput")
    wqT = nc.dram_tensor("wqT", [P, HT, DC], DT16, kind="ExternalInput")
    wkT = nc.dram_tensor("wkT", [P, HT, DC], DT16, kind="ExternalInput")
    wvT = nc.dram_tensor("wvT", [P, HT, DC], DT16, kind="ExternalInput")
    woT = nc.dram_tensor("woT", [P, HPC, H], DT16, kind="ExternalInput")
    out = nc.dram_tensor("out", [B * st_n, P, H], DT16, kind="ExternalOutput")

    with tile.TileContext(nc) as tc:
        with (
            tc.tile_pool(name="const", bufs=1) as constp,
            tc.tile_pool(name="persist", bufs=1) as persist,
        ):
            ones16 = constp.tile([P, 1], DT16)
            nc.vector.memset(ones16[:], 1.0)

            qT = persist.tile([P, HPC, bs], DT16)
            kT = persist.tile([P, HPC, bs], DT16)
            vsb = persist.tile([P, vt_n, DC], DT16)
            wo_sb = persist.tile([P, HPC, H], DT16)

            # evacuation helper: alternate DVE/ACT so neither paces the PE
            def evac(idx, dst, src):
                if idx % 2 == 0:
                    nc.scalar.copy(dst, src)
                else:
                    nc.vector.tensor_copy(dst, src)

            # ---------------- Phase 1: q/k/v projections ----------------
            with (
                tc.tile_pool(name="wpool", bufs=1) as wpool,
                tc.tile_pool(name="hpool", bufs=10) as hpool,
                tc.tile_pool(name="ppsum", bufs=2, space="PSUM") as ppsum,
                tc.tile_pool(name="vpsum", bufs=4, space="PSUM") as vpsum,
            ):
                wq_sb = wpool.tile([P, HT, DC], DT16, tag="wq")
                wk_sb = wpool.tile([P, HT, DC], DT16, tag="wk")
                wv_sb = wpool.tile([P, HT, DC], DT16, tag="wv")

                for sc in range(nsc):
                    quarters = []
                    for qf in range(NQT):
                        # interleave wq quarters with the first chunk's hsT
                        # quarters so the first matmul group starts after
                        # ~0.8 MB of DMA instead of 3 MB
                        if sc == 0:
                            _wsl = slice(qf * KOQ, (qf + 1) * KOQ)
                            nc.sync.dma_start(wq_sb[:, _wsl], wqT.ap()[:, _wsl])
                        hst = hpool.tile([P, KOQ, QC], DT16, tag="hst")
                        nc.sync.dma_start(hst[:], hsT.ap()[sc, qf])
                        quarters.append(hst)
                    if sc == 0:
                        nc.sync.dma_start(wk_sb[:], wkT.ap())
                        nc.sync.dma_start(wv_sb[:], wvT.ap())
                    if sc == 1:
                        # wo is phase-2-only; don't let it block phase-1 DMAs
                        nc.sync.dma_start(wo_sb[:], woT.ap())

                    def hq(ko):
                        return quarters[ko // KOQ][:, ko % KOQ]

                    ssl = slice(sc * QC, (sc + 1) * QC)
                    # chunk 0: run both wq groups before the wk groups so the
                    # PE never waits on the (later-issued) wk DMA
                    if sc == 0:
                        groups = [(h, w) for w in ("q", "k") for h in range(HPC)]
                    else:
                        groups = [(h, w) for h in range(HPC) for w in ("q", "k")]
                    ev = sc
                    for h, w in groups:
                        wsb, dstT = (wq_sb, qT) if w == "q" else (wk_sb, kT)
                        ps = ppsum.tile([P, QC], F32, tag="psqk")
                        for ko in range(HT):
                            nc.tensor.matmul(
                                ps[:],
                                wsb[:, ko, h * P : (h + 1) * P],
                                hq(ko),
                                start=(ko == 0),
                                stop=(ko == HT - 1),
                            )
                        evac(ev, dstT[:, h, ssl], ps[:])
                        ev += 1
                    # v: ko-outer over 4 concurrent PSUM groups so each hsT
                    # quarter is consumed once and can be recycled early
                    psvs = []
                    for st in range(QC // P):
                        psv = vpsum.tile([P, DC], F32, tag="psv")
                        psvs.append(psv)
                    for ko in range(HT):
                        for st in range(QC // P):
                            nc.tensor.matmul(
                                psvs[st][:],
                                hq(ko)[:, st * P : (st + 1) * P],
                                wv_sb[:, ko, :],
                                start=(ko == 0),
                                stop=(ko == HT - 1),
                            )
                    for st in range(QC // P):
                        evac(ev, vsb[:, sc * (QC // P) + st, :], psvs[st][:])
                        ev += 1

            # ---------------- Phase 2: attention + output projection ----------------
            with (
                tc.tile_pool(name="mpool", bufs=6) as mpool,
                tc.tile_pool(name="apool", bufs=2) as apool,
                tc.tile_pool(name="smpool", bufs=2) as smpool,
                tc.tile_pool(name="upool", bufs=2) as upool,
                tc.tile_pool(name="rpool", bufs=2) as rpool,
                tc.tile_pool(name="opool", bufs=3) as opool,
                tc.tile_pool(name="spsum", bufs=3, space="PSUM") as spsum,
                tc.tile_pool(name="upsum", bufs=2, space="PSUM") as upsum,
                tc.tile_pool(name="dpsum", bufs=1, space="PSUM") as dpsum,
                tc.tile_pool(name="opsum", bufs=2, space="PSUM") as opsum,
            ):
                units = [
                    (b, qq, h)
                    for b in range(B)
                    for qq in range(nq)
                    for h in range(HPC)
                ]
                nu = len(units)
                mslabs = {}
                aslabs = {}
                psus = {}
                psds = {}
                uTs = {}
                KH = kt_n // 2
                NDVE_MASK = 7  # mask-mult tiles on DVE; rest on GpSimd

                def mask_prefetch(i):
                    b, qq, h = units[i]
                    if h == 0 and (b, qq) not in mslabs:
                        halves = []
                        for mh in range(2):
                            ms = mpool.tile([P, KH, QC], DT16, tag="mslab")
                            nc.sync.dma_start(ms[:], maskT.ap()[b, qq, mh])
                            halves.append(ms)
                        mslabs[(b, qq)] = halves

                def qk_part(i, kt):
                    # scoresT k-tile matmul; exp(s+m) = exp(s)*exp(m): the
                    # ScalarE exp evacuates PSUM directly and the mask factor
                    # (host-precomputed exp(mask)) is applied as an fp16
                    # SBUF*SBUF multiply on DVE or GpSimd
                    b, qq, h = units[i]
                    if kt == 0:
                        asl = apool.tile([P, kt_n, QC], DT16, tag="aslab")
                        aslabs[i] = asl
                    asl = aslabs[i]
                    ms = mslabs[(b, qq)][kt // KH]
                    pss = spsum.tile([P, QC], F32, tag="pss")
                    nc.tensor.matmul(
                        pss[:],
                        kT[:, h, b * s + kt * P : b * s + (kt + 1) * P],
                        qT[:, h, b * s + qq * QC : b * s + (qq + 1) * QC],
                        start=True,
                        stop=True,
                    )
                    nc.scalar.activation(asl[:, kt], pss[:], EXP)
                    eng = nc.vector if kt < NDVE_MASK else nc.gpsimd
                    eng.tensor_tensor(asl[:, kt], asl[:, kt], ms[:, kt % KH], MULT)

                def pv_part(i, kt):
                    b, qq, h = units[i]
                    asl = aslabs[i]
                    if kt == 0:
                        psu = upsum.tile([P, QC], F32, tag="psu")
                        psus[i] = psu
                    nc.tensor.matmul(
                        psus[i][:],
                        vsb[:, b * kt_n + kt, h * P : (h + 1) * P],
                        asl[:, kt],
                        start=(kt == 0),
                        stop=(kt == kt_n - 1),
                    )

                def den_tree(i):
                    # DVE pairwise add halves the attn stream the PE has to
                    # re-read for the denominator ones-matmuls
                    asl = aslabs[i]
                    asum = smpool.tile([P, KH, QC], DT16, tag="asum")
                    nc.vector.tensor_tensor(
                        asum[:], asl[:, 0:KH], asl[:, KH : 2 * KH], ADD
                    )
                    return asum

                def den_mms(i, asum, j0, j1):
                    if j0 == 0:
                        psd = dpsum.tile([1, QC], F32, tag="psd")
                        psds[i] = psd
                    for j in range(j0, j1):
                        nc.tensor.matmul(
                            psds[i][:],
                            ones16[:],
                            asum[:, j],
                            start=(j == 0),
                            stop=(j == KH - 1),
                        )

                rbcs = {}

                def finish_recip(i):
                    # 1/denom: ~51-ULP DVE approx, replicated across
                    # partitions on the (currently idle) GpSimd. Emitted a
                    # couple of kt slots before finish_norm so the DVE never
                    # stalls waiting for the broadcast.
                    recip = rpool.tile([1, QC], F32, tag="recip")
                    nc.vector.reciprocal_approx_fast(out=recip[:], in_=psds.pop(i)[:])
                    rbc = rpool.tile([P, QC], F32, tag="rbc")
                    nc.gpsimd.partition_broadcast(rbc[:], recip[:])
                    rbcs[i] = rbc

                def finish_norm(i):
                    b, qq, h = units[i]
                    aslabs.pop(i)
                    if b not in uTs:
                        uTs[b] = upool.tile([P, HPC, s], DT16, tag="uT", name="uT")
                    nc.vector.tensor_tensor(
                        uTs[b][:, h, qq * QC : (qq + 1) * QC],
                        psus.pop(i)[:],
                        rbcs.pop(i)[:],
                        MULT,
                    )

                # out-projection work list: one (b, qq, st) row = 4 jc tiles;
                # each tile is a 2-matmul (head) PSUM accumulation. Emitted
                # interleaved into later units' kt loops, 2 rows per unit.
                out_rows = []
                ot_cur = [None]
                ev_ctr = [0]

                def outproj_row(b, qq, st):
                    uT_b = uTs[b]
                    stl = qq * (QC // P) + st
                    ot = opool.tile([P, H], DT16, tag="ot")
                    for jc in range(H // QC):
                        pso = opsum.tile([P, QC], F32, tag="pso")
                        for h in range(HPC):
                            nc.tensor.matmul(
                                pso[:],
                                uT_b[:, h, stl * P : (stl + 1) * P],
                                wo_sb[:, h, jc * QC : (jc + 1) * QC],
                                start=(h == 0),
                                stop=(h == HPC - 1),
                            )
                        jsl = slice(jc * QC, (jc + 1) * QC)
                        # mostly DVE: ScalarE's exp budget only affords ~1/6
                        if ev_ctr[0] % 6 == 5:
                            nc.scalar.copy(ot[:, jsl], pso[:])
                        else:
                            nc.vector.tensor_copy(ot[:, jsl], pso[:])
                        ev_ctr[0] += 1
                    nc.sync.dma_start(out.ap()[b * st_n + stl], ot[:])
                    if qq == nq - 1 and st == (QC // P) - 1:
                        uTs.pop(b)

                # software pipeline: period i interleaves QK(i+1) with PV(i)
                # at k-tile granularity. Cross-engine consumers are emitted
                # several kt slots after their producers so no engine's
                # in-order queue ever blocks the PE: recip/broadcast of
                # unit i-1 at kt 2, the DVE denominator tree at kt 3, the
                # uT normalize at kt 4, out-projection rows at kt 6/10, and
                # the denominator ones-matmuls at kt 10..13 (by when the
                # tree has long drained from the DVE queue).
                mask_prefetch(0)
                mask_prefetch(1)
                for kt in range(kt_n):
                    qk_part(0, kt)
                for i in range(nu):
                    if i + 1 < nu:
                        mask_prefetch(i + 1)
                    if i + 2 < nu:
                        mask_prefetch(i + 2)
                    asum = [None]
                    for kt in range(kt_n):
                        if i + 1 < nu:
                            qk_part(i + 1, kt)
                        pv_part(i, kt)
                        if kt == 2 and i >= 1:
                            finish_recip(i - 1)
                        if kt == 3:
                            asum[0] = den_tree(i)
                        if kt == 4 and i >= 1:
                            finish_norm(i - 1)
                            b1, qq1, h1 = units[i - 1]
                            if h1 == HPC - 1:
                                out_rows += [
                                    (b1, qq1, st) for st in range(QC // P)
                                ]
                        if kt in (6, 10) and out_rows:
                            outproj_row(*out_rows.pop(0))
                        if 10 <= kt <= 13:
                            den_mms(i, asum[0], (kt - 10) * 2, (kt - 9) * 2)
                    b, qq, h = units[i]
                # tail: last unit's normalize + the final pair's
                # out-projection have nothing to hide behind
                finish_recip(nu - 1)
                finish_norm(nu - 1)
                out_rows += [(B - 1, nq - 1, st) for st in range(QC // P)]
                for row in out_rows:
                    outproj_row(*row)

    nc.compile()
    return nc


def make_in_maps(hs, mask, Wq, Wk, Wv, Wo):
    """Host-side prep: transpose/shard/pre-tile the full inputs per core."""
    bs = hs.shape[0] * hs.shape[1]
    f16 = np.float16
    # hsT tiled [sc, qf, p, ko, t]: row (qf*KOQ+ko)*P + p, col sc*QC + t
    hsT = np.ascontiguousarray(
        hs.reshape(bs, H).T.reshape(NQT, KOQ, P, bs // QC, QC)
        .transpose(3, 0, 2, 1, 4)
    ).astype(f16)
    # mask tiled [b, qq, half, p, kt, q]: k = (half*KH+kt)*P + p, q = qq*QC + q0
    KH = (S // P) // 2
    m = np.exp(np.asarray(mask[:, 0]).transpose(0, 2, 1))  # [b, k, q]
    maskT = np.ascontiguousarray(
        m.reshape(B, 2, KH, P, S // QC, QC).transpose(0, 4, 1, 3, 2, 5)
    ).astype(f16)

    def wtile(w):  # [2048, DC] -> [p, o, d]
        return np.ascontiguousarray(
            w.reshape(HT, P, DC).transpose(1, 0, 2)
        ).astype(f16)

    in_maps = []
    for c in range(NCORES):
        sl = slice(c * DC, (c + 1) * DC)
        in_maps.append(
            {
                "hsT": hsT,
                "maskT": maskT,
                "wqT": wtile(np.asarray(Wq[sl] * SCALE).T),
                "wkT": wtile(np.asarray(Wk[sl]).T),
                "wvT": wtile(np.asarray(Wv[sl]).T),
                "woT": np.ascontiguousarray(
                    np.asarray(Wo[:, sl]).T.reshape(HPC, P, H).transpose(1, 0, 2)
                ).astype(f16),
            }
        )
    return in_maps


_NC_CACHE = {}


def get_nc(s=S):
    if s not in _NC_CACHE:
        _NC_CACHE[s] = build_attention_nc(s)
    return _NC_CACHE[s]


def run(hs, mask, Wq, Wk, Wv, Wo, trace=False, trace_kwargs=None):
    s = hs.shape[1]
    nc = get_nc(s)
    in_maps = make_in_maps(hs, mask, Wq, Wk, Wv, Wo)
    res = run_bass_kernel_spmd(
        nc,
        in_maps,
        core_ids=list(range(NCORES)),
        trace=trace,
        **(trace_kwargs or {}),
    )
    parts = np.stack([r["out"] for r in res.results])  # [8, 32, 128, 2048] fp16
    full = parts.astype(np.float32).sum(axis=0)
    return full.reshape(hs.shape[0], s, H), res


def kernel(hidden_states, attention_mask, Wq, Wk, Wv, Wo):
    hs = np.asarray(hidden_states, dtype=np.float32)
    mask = np.asarray(attention_mask, dtype=np.float32)
    Wq = np.asarray(Wq, dtype=np.float32)
    Wk = np.asarray(Wk, dtype=np.float32)
    Wv = np.asarray(Wv, dtype=np.float32)
    Wo = np.asarray(Wo, dtype=np.float32)
    out, _ = run(hs, mask, Wq, Wk, Wv, Wo)
    return out


# revision 12
# speedup vs baseline: 1.5374x; 1.5374x over previous
"""Trainium2 Bass kernel for nn_DualWeightAttention (B=2, S=2048, H=2048, 16 heads).

Sharding: tensor-parallel over heads — 2 heads per core on 8 cores.
Each core computes q/k/v projections for its 2 heads, attention for those
heads (both batches), and a partial output projection against its 256-row
slice of Wo.T. The 8 partial [4096, 2048] fp16 outputs are summed on the host.

On-chip layouts (per core), fp16 operands (same PE rate as bf16, 8x finer
mantissa; every tensor here is O(1)-scaled so range is ample):
  qT, kT  [128(d), head, B*S]  fp16  (head dim on partitions)
  v       [128(s), tile, 256]  fp16  (seq on partitions)
  scoresT [128(k), q]          psum f32 = (kT k-tile).T @ qT chunk
  attn_u  [128(k), kt, 512]    fp16  = exp(scoresT) * exp(maskT)   (host
                                       precomputes exp(mask); ScalarE's exp
                                       evacuates PSUM; the mask factor is an
                                       fp16 SBUF*SBUF multiply split across
                                       DVE and GpSimd so neither paces)
  asum8   [128(k), 8, 512]     fp16  = attn_u[:, 0:8] + attn_u[:, 8:16] (DVE)
  uT      [128(d), head, S]    fp16  = ((attn_u @ v)^T) * 1/denom
  out     [128(s), 2048]       fp16  = uT.T @ WoT-slice (2-head accumulate)

Softmax is unnormalized: the denominator is a DVE pairwise add (halves the
PE ones-matmul stream) followed by 8 accumulating ones-vector matmuls,
1/denom via a DVE approx reciprocal broadcast across partitions on GpSimd,
applied while evacuating the PV accumulator.

Phase-2 emission is software-pipelined at k-tile granularity: period i
interleaves QK(i+1) matmuls with PV(i) matmuls, the denominator matmuls of
unit i (kt 2..5), and the out-projection tiles of the previously finished
q-chunk pair (kt 6..13, half a pair per unit) — the PE never sits behind
the reciprocal/normalize chain and stays ~95% busy.

DMA: all DRAM tensors are host-pre-tiled so every descriptor is a 2-8 KiB
contiguous run (hsT [sc,qf,p,ko,t], mask [b,qq,half,p,kt,q], out
[st,p,j]); output is stored fp16 (error ~5e-4, far inside the 2e-2 gate).
"""

import numpy as np

import concourse.mybir as mybir
import concourse.tile as tile
from concourse import bacc
from concourse.bass_utils import run_bass_kernel_spmd

P = 128
B = 2
S = 2048
H = 2048
NH = 16
HD = 128
NCORES = 8
HPC = NH // NCORES  # heads per core
DC = HPC * HD       # d-columns per core
QC = 512            # q-chunk (matmul moving free dim)
HT = H // P         # contraction tiles for projections
NQT = 4             # hsT quarters per s-chunk
KOQ = HT // NQT
SCALE = 1.0 / float(np.sqrt(HD))

F32 = mybir.dt.float32
DT16 = mybir.dt.float16

EXP = mybir.ActivationFunctionType.Exp
ADD = mybir.AluOpType.add
MULT = mybir.AluOpType.mult


def build_attention_nc(s=S):
    bs = B * s
    kt_n = s // P   # k tiles per batch
    nq = s // QC    # q chunks per batch
    st_n = s // P   # s tiles per batch (out projection)
    vt_n = bs // P  # v tiles (both batches)
    nsc = bs // QC  # s-chunks (projection)

    nc = bacc.Bacc("TRN2", target_bir_lowering=False, debug=False, num_devices=NCORES)
    # host-pre-tiled DRAM layouts: every per-partition run is contiguous
    hsT = nc.dram_tensor("hsT", [nsc, NQT, P, KOQ, QC], DT16, kind="ExternalInput")
    maskT = nc.dram_tensor("maskT", [B, nq, 2, P, kt_n // 2, QC], DT16,
                           kind="ExternalInput")
    wqT = nc.dram_tensor("wqT", [P, HT, DC], DT16, kind="ExternalInput")
    wkT = nc.dram_tensor("wkT", [P, HT, DC], DT16, kind="ExternalInput")
    wvT = nc.dram_tensor("wvT", [P, HT, DC], DT16, kind="ExternalInput")
    woT = nc.dram_tensor("woT", [P, HPC, H], DT16, kind="ExternalInput")
    out = nc.dram_tensor("out", [B * st_n, P, H], DT16, kind="ExternalOutput")

    with tile.TileContext(nc) as tc:
        with (
            tc.tile_pool(name="const", bufs=1) as constp,
            tc.tile_pool(name="persist", bufs=1) as persist,
        ):
            ones16 = constp.tile([P, 1], DT16)
            nc.vector.memset(ones16[:], 1.0)

            qT = persist.tile([P, HPC, bs], DT16)
            kT = persist.tile([P, HPC, bs], DT16)
            vsb = persist.tile([P, vt_n, DC], DT16)
            wo_sb = persist.tile([P, HPC, H], DT16)

            # evacuation helper: alternate DVE/ACT so neither paces the PE
            def evac(idx, dst, src):
                if idx % 2 == 0:
                    nc.scalar.copy(dst, src)
                else:
                    nc.vector.tensor_copy(dst, src)

            # ---------------- Phase 1: q/k/v projections ----------------
            with (
                tc.tile_pool(name="wpool", bufs=1) as wpool,
                tc.tile_pool(name="hpool", bufs=10) as hpool,
                tc.tile_pool(name="ppsum", bufs=2, space="PSUM") as ppsum,
                tc.tile_pool(name="vpsum", bufs=4, space="PSUM") as vpsum,
            ):
                wq_sb = wpool.tile([P, HT, DC], DT16, tag="wq")
                wk_sb = wpool.tile([P, HT, DC], DT16, tag="wk")
                wv_sb = wpool.tile([P, HT, DC], DT16, tag="wv")

                for sc in range(nsc):
                    quarters = []
                    for qf in range(NQT):
                        # interleave wq quarters with the first chunk's hsT
                        # quarters so the first matmul group starts after
                        # ~0.8 MB of DMA instead of 3 MB
                        if sc == 0:
                            _wsl = slice(qf * KOQ, (qf + 1) * KOQ)
                            nc.sync.dma_start(wq_sb[:, _wsl], wqT.ap()[:, _wsl])
                        hst = hpool.tile([P, KOQ, QC], DT16, tag="hst")
                        nc.sync.dma_start(hst[:], hsT.ap()[sc, qf])
                        quarters.append(hst)
                    if sc == 0:
                        nc.sync.dma_start(wk_sb[:], wkT.ap())
                        nc.sync.dma_start(wv_sb[:], wvT.ap())
                    if sc == 1:
                        # wo is phase-2-only; don't let it block phase-1 DMAs
                        nc.sync.dma_start(wo_sb[:], woT.ap())

                    def hq(ko):
                        return quarters[ko // KOQ][:, ko % KOQ]

                    ssl = slice(sc * QC, (sc + 1) * QC)
                    # chunk 0: run both wq groups before the wk groups so the
                    # PE never waits on the (later-issued) wk DMA
                    if sc == 0:
                        groups = [(h, w) for w in ("q", "k") for h in range(HPC)]
                    else:
                        groups = [(h, w) for h in range(HPC) for w in ("q", "k")]
                    ev = sc
                    for h, w in groups:
                        wsb, dstT = (wq_sb, qT) if w == "q" else (wk_sb, kT)
                        ps = ppsum.tile([P, QC], F32, tag="psqk")
                        for ko in range(HT):
                            nc.tensor.matmul(
                                ps[:],
                                wsb[:, ko, h * P : (h + 1) * P],
                                hq(ko),
                                start=(ko == 0),
                                stop=(ko == HT - 1),
                            )
                        evac(ev, dstT[:, h, ssl], ps[:])
                        ev += 1
                    # v: ko-outer over 4 concurrent PSUM groups so each hsT
                    # quarter is consumed once and can be recycled early
                    psvs = []
                    for st in range(QC // P):
                        psv = vpsum.tile([P, DC], F32, tag="psv")
                        psvs.append(psv)
                    for ko in range(HT):
                        for st in range(QC // P):
                            nc.tensor.matmul(
                                psvs[st][:],
                                hq(ko)[:, st * P : (st + 1) * P],
                                wv_sb[:, ko, :],
                                start=(ko == 0),
                                stop=(ko == HT - 1),
                            )
                    for st in range(QC // P):
                        evac(ev, vsb[:, sc * (QC // P) + st, :], psvs[st][:])
                        ev += 1

            # ---------------- Phase 2: attention + output projection ----------------
            with (
                tc.tile_pool(name="mpool", bufs=6) as mpool,
                tc.tile_pool(name="apool", bufs=2) as apool,
                tc.tile_pool(name="upool", bufs=2) as upool,
                tc.tile_pool(name="rpool", bufs=2) as rpool,
                tc.tile_pool(name="opool", bufs=3) as opool,
                tc.tile_pool(name="spsum", bufs=3, space="PSUM") as spsum,
                tc.tile_pool(name="upsum", bufs=2, space="PSUM") as upsum,
                tc.tile_pool(name="dpsum", bufs=1, space="PSUM") as dpsum,
                tc.tile_pool(name="opsum", bufs=2, space="PSUM") as opsum,
            ):
                units = [
                    (b, qq, h)
                    for b in range(B)
                    for qq in range(nq)
                    for h in range(HPC)
                ]
                nu = len(units)
                mslabs = {}
                aslabs = {}
                psus = {}
                psds = {}
                uTs = {}
                KH = kt_n // 2

                def mask_prefetch(i):
                    b, qq, h = units[i]
                    if h == 0 and (b, qq) not in mslabs:
                        halves = []
                        for mh in range(2):
                            ms = mpool.tile([P, KH, QC], DT16, tag="mslab")
                            nc.sync.dma_start(ms[:], maskT.ap()[b, qq, mh])
                            halves.append(ms)
                        mslabs[(b, qq)] = halves

                def qk_part(i, kt):
                    # scoresT k-tile matmul; exp(s+m) = exp(s)*exp(m): the
                    # ScalarE exp evacuates PSUM directly and the mask factor
                    # (host-precomputed exp(mask)) is applied as an fp16
                    # SBUF*SBUF multiply on DVE or GpSimd
                    b, qq, h = units[i]
                    if kt == 0:
                        asl = apool.tile([P, kt_n, QC], DT16, tag="aslab")
                        aslabs[i] = asl
                    asl = aslabs[i]
                    ms = mslabs[(b, qq)][kt // KH]
                    pss = spsum.tile([P, QC], F32, tag="pss")
                    nc.tensor.matmul(
                        pss[:],
                        kT[:, h, b * s + kt * P : b * s + (kt + 1) * P],
                        qT[:, h, b * s + qq * QC : b * s + (qq + 1) * QC],
                        start=True,
                        stop=True,
                    )
                    nc.scalar.activation(asl[:, kt], pss[:], EXP)
                    nc.vector.tensor_tensor(
                        asl[:, kt], asl[:, kt], ms[:, kt % KH], MULT
                    )

                def pv_part(i, kt):
                    b, qq, h = units[i]
                    asl = aslabs[i]
                    if kt == 0:
                        psu = upsum.tile([P, QC], F32, tag="psu")
                        psus[i] = psu
                    nc.tensor.matmul(
                        psus[i][:],
                        vsb[:, b * kt_n + kt, h * P : (h + 1) * P],
                        asl[:, kt],
                        start=(kt == 0),
                        stop=(kt == kt_n - 1),
                    )

                def den_mm(i, kt):
                    # denominator ones-matmul for k-tile kt, spread 1/kt
                    # through the unit. Tile 0 is deferred to ride with tile
                    # 15: the psd bank (dpsum bufs=1) is only freed by the
                    # previous unit's reciprocal, which is emitted at this
                    # unit's top — one kt of slack covers the handoff.
                    asl = aslabs[i]
                    if kt == 0:
                        return
                    if kt == 1:
                        psds[i] = dpsum.tile([1, QC], F32, tag="psd", name="psd")
                    for t in ((kt, 0) if kt == kt_n - 1 else (kt,)):
                        nc.tensor.matmul(
                            psds[i][:],
                            ones16[:],
                            asl[:, t],
                            start=(kt == 1 and t == kt),
                            stop=(t == 0),
                        )

                rbcs = {}

                def finish_recip(i):
                    # 1/denom: ~51-ULP DVE approx, replicated across
                    # partitions on the (currently idle) GpSimd. Emitted a
                    # couple of kt slots before finish_norm so the DVE never
                    # stalls waiting for the broadcast.
                    recip = rpool.tile([1, QC], F32, tag="recip")
                    nc.vector.reciprocal_approx_fast(out=recip[:], in_=psds.pop(i)[:])
                    rbc = rpool.tile([P, QC], F32, tag="rbc")
                    nc.gpsimd.partition_broadcast(rbc[:], recip[:])
                    rbcs[i] = rbc

                def finish_norm(i):
                    b, qq, h = units[i]
                    aslabs.pop(i)
                    if b not in uTs:
                        uTs[b] = upool.tile([P, HPC, s], DT16, tag="uT", name="uT")
                    nc.vector.tensor_tensor(
                        uTs[b][:, h, qq * QC : (qq + 1) * QC],
                        psus.pop(i)[:],
                        rbcs.pop(i)[:],
                        MULT,
                    )

                # out-projection work list: one (b, qq, st) row = 4 jc tiles;
                # each tile is a 2-matmul (head) PSUM accumulation. Emitted
                # interleaved into later units' kt loops, 2 rows per unit.
                out_rows = []
                ot_cur = [None]
                ev_ctr = [0]

                def outproj_row(b, qq, st):
                    uT_b = uTs[b]
                    stl = qq * (QC // P) + st
                    ot = opool.tile([P, H], DT16, tag="ot")
                    for jc in range(H // QC):
                        pso = opsum.tile([P, QC], F32, tag="pso")
                        for h in range(HPC):
                            nc.tensor.matmul(
                                pso[:],
                                uT_b[:, h, stl * P : (stl + 1) * P],
                                wo_sb[:, h, jc * QC : (jc + 1) * QC],
                                start=(h == 0),
                                stop=(h == HPC - 1),
                            )
                        jsl = slice(jc * QC, (jc + 1) * QC)
                        # mostly DVE: ScalarE's exp budget only affords ~1/4
                        if ev_ctr[0] % 4 == 3:
                            nc.scalar.copy(ot[:, jsl], pso[:])
                        else:
                            nc.vector.tensor_copy(ot[:, jsl], pso[:])
                        ev_ctr[0] += 1
                    nc.sync.dma_start(out.ap()[b * st_n + stl], ot[:])
                    if qq == nq - 1 and st == (QC // P) - 1:
                        uTs.pop(b)

                # software pipeline: period i interleaves QK(i+1) with PV(i)
                # and the denominator ones-matmul of tile kt. Cross-engine
                # consumers sit a few kt slots behind their producers so no
                # in-order queue blocks the PE: recip/broadcast of unit i-1
                # at the unit top (GpSimd is idle then), the uT normalize at
                # kt 2, out-projection rows of the finished pair at kt 6/12.
                mask_prefetch(0)
                mask_prefetch(1)
                for kt in range(kt_n):
                    qk_part(0, kt)
                for i in range(nu):
                    if i + 1 < nu:
                        mask_prefetch(i + 1)
                    if i + 2 < nu:
                        mask_prefetch(i + 2)
                    if i >= 1:
                        finish_recip(i - 1)
                    for kt in range(kt_n):
                        if i + 1 < nu:
                            qk_part(i + 1, kt)
                        pv_part(i, kt)
                        den_mm(i, kt)
                        if kt == 2 and i >= 1:
                            finish_norm(i - 1)
                            b1, qq1, h1 = units[i - 1]
                            if h1 == HPC - 1:
                                out_rows += [
                                    (b1, qq1, st) for st in range(QC // P)
                                ]
                        if kt in (6, 12) and out_rows:
                            outproj_row(*out_rows.pop(0))
                # tail: last unit's normalize + the final pair's
                # out-projection have nothing to hide behind
                finish_recip(nu - 1)
                finish_norm(nu - 1)
                out_rows += [(B - 1, nq - 1, st) for st in range(QC // P)]
                for row in out_rows:
                    outproj_row(*row)

    nc.compile()
    return nc


def make_in_maps(hs, mask, Wq, Wk, Wv, Wo):
    """Host-side prep: transpose/shard/pre-tile the full inputs per core."""
    bs = hs.shape[0] * hs.shape[1]
    f16 = np.float16
    # hsT tiled [sc, qf, p, ko, t]: row (qf*KOQ+ko)*P + p, col sc*QC + t
    hsT = np.ascontiguousarray(
        hs.reshape(bs, H).T.reshape(NQT, KOQ, P, bs // QC, QC)
        .transpose(3, 0, 2, 1, 4)
    ).astype(f16)
    # mask tiled [b, qq, half, p, kt, q]: k = (half*KH+kt)*P + p, q = qq*QC + q0
    KH = (S // P) // 2
    m = np.exp(np.asarray(mask[:, 0]).transpose(0, 2, 1))  # [b, k, q]
    maskT = np.ascontiguousarray(
        m.reshape(B, 2, KH, P, S // QC, QC).transpose(0, 4, 1, 3, 2, 5)
    ).astype(f16)

    def wtile(w):  # [2048, DC] -> [p, o, d]
        return np.ascontiguousarray(
            w.reshape(HT, P, DC).transpose(1, 0, 2)
        ).astype(f16)

    in_maps = []
    for c in range(NCORES):
        sl = slice(c * DC, (c + 1) * DC)
        in_maps.append(
            {
                "hsT": hsT,
                "maskT": maskT,
                "wqT": wtile(np.asarray(Wq[sl] * SCALE).T),
                "wkT": wtile(np.asarray(Wk[sl]).T),
                "wvT": wtile(np.asarray(Wv[sl]).T),
                "woT": np.ascontiguousarray(
                    np.asarray(Wo[:, sl]).T.reshape(HPC, P, H).transpose(1, 0, 2)
                ).astype(f16),
            }
        )
    return in_maps


_NC_CACHE = {}


def get_nc(s=S):
    if s not in _NC_CACHE:
        _NC_CACHE[s] = build_attention_nc(s)
    return _NC_CACHE[s]


def run(hs, mask, Wq, Wk, Wv, Wo, trace=False, trace_kwargs=None):
    s = hs.shape[1]
    nc = get_nc(s)
    in_maps = make_in_maps(hs, mask, Wq, Wk, Wv, Wo)
    res = run_bass_kernel_spmd(
        nc,
        in_maps,
        core_ids=list(range(NCORES)),
        trace=trace,
        **(trace_kwargs or {}),
    )
    parts = np.stack([r["out"] for r in res.results])  # [8, 32, 128, 2048] fp16
    full = parts.astype(np.float32).sum(axis=0)
    return full.reshape(hs.shape[0], s, H), res


def kernel(hidden_states, attention_mask, Wq, Wk, Wv, Wo):
    hs = np.asarray(hidden_states, dtype=np.float32)
    mask = np.asarray(attention_mask, dtype=np.float32)
    Wq = np.asarray(Wq, dtype=np.float32)
    Wk = np.asarray(Wk, dtype=np.float32)
    Wv = np.asarray(Wv, dtype=np.float32)
    Wo = np.asarray(Wo, dtype=np.float32)
    out, _ = run(hs, mask, Wq, Wk, Wv, Wo)
    return out
